# revision 33
# baseline (speedup 1.0000x reference)
"""Trainium2 Bass kernel for CronRootAttention (sparse attention).

Shapes (hardcoded): B=2 H=16 S=4096 D=128, W=64, NB=R=64.
Sharding: fused B*H=32 axis split across 8 cores (4 slices/core).

Design (transposed scores + multiplicative masks + paired tiles), per
(b,h) slice, per 128-query tile i (group g = i//4 covers 512 queries):

  scores computed TRANSPOSED: S[key, query] via key-stationary QK
  matmuls so exp(S) is directly the stationary operand for PV (no PE
  transposes anywhere).

  Local scores for a PAIR of tiles (2t, 2t+1) share ONE PSUM bank
  [128, 384] (2x192 fp32 cols = 1536B <= 2KB bank):
    cols   0:128  A(2t):   keys 256t-64 .. 256t+64
    cols 128:192  B(2t):   keys 256t+64 .. 256t+128  (rows 64:128 junk)
    cols 192:320  A(2t+1), cols 320:384  B(2t+1)
  B(i) reuses the identical 128-col stationary of A(i+1) (adjacent in
  the PE queue) so its weight load hides.  Strided/relay scores per
  GROUP in PSUM Sc [128, 512]: one matmul, stationary kTsr (128
  interleaved strided/relay keys), moving qT[:, 512g:512g+512].

  NO additive -1e30 mask matmuls on the PE.  exp() runs on raw scores
  (stale PSUM is bounded: banks are memset once, then only ever hold
  old scores, so exp stays finite) and the band/causal masks are
  applied POST-exp as multiplicative 0/1 bf16 masks: ONE GpSimd
  tensor_tensor per pair ([128,384]), ONE Vector tensor_tensor per
  group ([128,512]).  Zeroed p rows contribute nothing to PV numerator
  or the ones-column denominator.

  PV per tile (3 matmuls into O [128, 129]): p regions stationary,
  moving v tiles carry a ones-column so O[:,128] = softmax denominator.
  The B-part PV uses the widened stationary p_abm[0:64, base+64:
  base+192] (full-array 128-col load; the extra columns are exactly the
  always-masked part of the A band so output rows 0:64 get +=0).
  DVE reciprocal + per-partition scale -> bf16 out column block; output
  DMA per 8 tiles from a contiguous [128, NT*128] SBUF accumulator
  (dram layout [slice, partition, tile*128+d]; host transposes).

  PSUM banks: Sa-pairs x3 + Sc x2 + O x3 = 8.

  Schedule: ONE flat software pipeline across all 4 slices (no
  inter-slice seams, keeps the PE p-state ramped): per step emit
  B(i)+A(i+1) (+ Cq at group boundaries), finish_pair on odd steps,
  PV lagging LAG=4 steps.  DMA: bulk streams chunked on the Sync DGE
  queue; latency-critical kTsr/masks on the Scalar DGE queue; the big
  C-mask tail is emitted late so it queues behind the first
  activations; input chunk 0 sized so compute starts ~2 DMAs in.
"""

import numpy as np
import ml_dtypes

import concourse.bass as bass
import concourse.bacc as bacc
import concourse.tile as tile
from concourse import mybir
from concourse.bass_utils import run_bass_kernel_spmd

BF16 = ml_dtypes.bfloat16
B, H, S, D = 2, 16, 4096, 128
W = 64
NB = S // W          # 64
R = NB               # 64
NCORES = 8
SLICES = B * H // NCORES   # 4
NT = S // 128        # 32 query tiles per slice
GT = 4               # tiles per strided-score group
NG = NT // GT        # 8 groups per slice
SCALE = 1.0 / np.sqrt(np.float32(D))
DV = D + 1           # v columns + ones column
NVT = S // 128 + 1   # 33 shifted v tiles

_prog_cache = {}
USE_MERGE = False
LAG = 4
WBUFS = 5
GBUFS = 3


def _build_consts():
    c = np.arange(128)[:, None]   # partition = key index within region
    j = np.arange(128)[None, :]   # col = query index within tile
    # AB mask (i>=1), multiplicative:
    #  A cols 0:128: key = 128i-64+c, query m = 128i+j: valid j+1<=c<=j+64
    #  B cols 128:192: key = 128i+64+c (c<64), query j'=j-64: valid c<=j'
    mA = ((c >= j + 1) & (c <= j + 64)).astype(np.float32)
    j2 = np.arange(64)[None, :]
    mB = ((c < 64) & (c <= j2)).astype(np.float32)
    mAB = np.concatenate([mA, mB], axis=1)             # [128, 192]
    # i=0 variant: key = c, query m = j: valid j-63<=c<=j; B region zero
    mAB0 = np.concatenate(
        [((c <= j) & (c >= j - 63)).astype(np.float32),
         np.zeros((128, 64), np.float32)], axis=1)
    # paired masks for the 2-tile Sa banks
    mABp = np.concatenate([mAB, mAB], axis=1)          # [128, 384]
    mABp0 = np.concatenate([mAB0, mAB], axis=1)
    # C masks per group g: [128, 512]; row 2s = strided key s (pos 64s),
    # row 2s+1 = relay s (block end 64s+63); query m = 512g + q.
    # valid strided: 64s < max(m-63,0); valid relay: 64s+63 < max(m-63,0)
    mC = np.zeros((NG, 128, 512), np.float32)
    s_ = np.arange(64)[:, None]
    for g in range(NG):
        m = (512 * g + np.arange(512))[None, :]
        ls = np.maximum(m - 63, 0)
        mC[g, 0::2, :] = (64 * s_ < ls).astype(np.float32)
        mC[g, 1::2, :] = (64 * s_ + 63 < ls).astype(np.float32)
    mCg = mC.transpose(1, 0, 2).reshape(128, NG * 512)  # [128, 8*512]
    return (mABp.astype(BF16), mABp0.astype(BF16),
            np.ascontiguousarray(mCg).astype(BF16))


def build_program():
    if "nc" in _prog_cache:
        return _prog_cache["nc"]
    dt = mybir.dt
    nc = bacc.Bacc("TRN2", target_bir_lowering=False, debug=False)

    qT_d = nc.declare_dram_parameter("qT", [SLICES, 128, S], dt.bfloat16, isOutput=False)
    kT_d = nc.declare_dram_parameter("kT", [SLICES, 128, S], dt.bfloat16, isOutput=False)
    vsh_d = nc.declare_dram_parameter("vsh", [SLICES, 128, NVT * DV], dt.bfloat16, isOutput=False)
    kTsr_d = nc.declare_dram_parameter("kTsr", [SLICES, 128, 128], dt.bfloat16, isOutput=False)
    vnr_d = nc.declare_dram_parameter("vnr", [SLICES, 128, 2 * DV], dt.bfloat16, isOutput=False)
    mABp_d = nc.declare_dram_parameter("mABp", [128, 384], dt.bfloat16, isOutput=False)
    mABp0_d = nc.declare_dram_parameter("mABp0", [128, 384], dt.bfloat16, isOutput=False)
    mCg_d = nc.declare_dram_parameter("mCg", [128, NG * 512], dt.bfloat16, isOutput=False)
    # out stored [slice, partition(=query%128), tile*128+d]; host transposes
    out_d = nc.declare_dram_parameter("out", [SLICES, 128, NT * D], dt.bfloat16, isOutput=True)

    from contextlib import ExitStack
    with tile.TileContext(nc) as tc, ExitStack() as ctx:
        cpool = ctx.enter_context(tc.tile_pool(name="consts", bufs=1))
        # paired AB masks: [tile 2t | tile 2t+1], 384 cols each
        mABp = cpool.tile([128, 384], dt.bfloat16, tag="mABp")
        mABp0 = cpool.tile([128, 384], dt.bfloat16, tag="mABp0")
        mCg = cpool.tile([128, NG * 512], dt.bfloat16, tag="mCg")
        # const DMAs are issued inside slice 0's prologue (critical first)

        spool = ctx.enter_context(tc.tile_pool(name="slice_in", bufs=2))
        pscores = ctx.enter_context(tc.tile_pool(name="pscores", bufs=3, space="PSUM"))
        pcpool = ctx.enter_context(tc.tile_pool(name="pcscores", bufs=2, space="PSUM"))
        pout = ctx.enter_context(tc.tile_pool(name="pout", bufs=3, space="PSUM"))
        wpool = ctx.enter_context(tc.tile_pool(name="work", bufs=WBUFS))
        gpool = ctx.enter_context(tc.tile_pool(name="gwork", bufs=GBUFS))
        opool = ctx.enter_context(tc.tile_pool(name="outacc", bufs=2))

        # one-time: clear the Sa banks so first-use stale PSUM can't be huge
        for z in range(3):
            zt = pscores.tile([128, 384], dt.float32, tag="scores")
            nc.vector.memset(zt[:], 0.0)

        # engine warm-ups: trigger the GpSimd tensor-op library load and the
        # first-use costs of both tensor_tensor paths during the startup DMA
        # window instead of on the pipeline's critical path
        wu_a = cpool.tile([128, 8], dt.bfloat16, tag="wu_a")
        wu_b = cpool.tile([128, 8], dt.bfloat16, tag="wu_b")
        wu_c = cpool.tile([128, 8], dt.bfloat16, tag="wu_c")
        wu_d = cpool.tile([128, 8], dt.bfloat16, tag="wu_d")
        nc.gpsimd.memset(wu_a[:], 1.0)
        nc.vector.memset(wu_b[:], 1.0)
        nc.gpsimd.tensor_tensor(wu_c[:], wu_a[:], wu_a[:], mybir.AluOpType.mult)
        nc.vector.tensor_tensor(wu_d[:], wu_b[:], wu_b[:], mybir.AluOpType.mult)

        state = {}
        gstate = {}
        cur = {}

        def cgroup(g):
            Sc = pcpool.tile([128, 512], dt.float32, tag="cscores")
            nc.tensor.matmul(Sc[:, :], cur["kTsr"][:, 0:128],
                             cur["qT"][:, 512 * g:512 * (g + 1)],
                             start=True, stop=True, skip_group_check=True)
            pc = gpool.tile([128, 512], dt.bfloat16, tag="pc")
            nc.scalar.activation(pc[:, :], Sc[:, :],
                                 mybir.ActivationFunctionType.Exp, scale=float(SCALE))
            pcm = gpool.tile([128, 512], dt.bfloat16, tag="pcm")
            nc.vector.tensor_tensor(pcm[:, :], pc[:, :],
                                    mCg[:, 512 * g:512 * (g + 1)],
                                    mybir.AluOpType.mult)
            gstate[g] = pcm

        sa_pairs = {}

        def emit_a(i):
            # A scores for tile i into the pair bank (pair = tiles 2t,2t+1
            # sharing one PSUM bank: tile cols at 192*(i&1)).  i>=1 shares
            # its stationary kT[:, 128i-64:128i+64] with the preceding
            # emit_b(i-1) (identical weights AP back-to-back).
            if i % 2 == 0:
                Sa = pscores.tile([128, 384], dt.float32, tag="scores")
                sa_pairs[i // 2] = Sa
            Sa = sa_pairs[i // 2]
            base = 192 * (i % 2)
            qTi = cur["qT"][:, 128 * i:128 * (i + 1)]
            kTa = cur["kT"][:, 0:128] if i == 0 else \
                cur["kT"][:, 128 * i - 64:128 * i + 64]
            # start=True only on the pair's first write (clears bank)
            nc.tensor.matmul(Sa[:, base:base + 128], kTa, qTi,
                             start=(i % 2 == 0), stop=False,
                             skip_group_check=True)

        def emit_ab(i):
            # even i: ONE matmul for B_i + A_{i+1} (one weight load).  Their
            # movings are adjacent qT cols 128i+64:128i+256 and the outputs
            # are adjacent PSUM cols 128:320 of the shared pair bank.  Output
            # cols 128:192 = B_i (rows 64:128 garbage, masked post-exp); for
            # i=0 the whole B region is garbage (masked by mABp0).
            Sa = sa_pairs[i // 2]
            nc.tensor.matmul(Sa[:, 128:320],
                             cur["kT"][:, 128 * i + 64:128 * i + 192],
                             cur["qT"][:, 128 * i + 64:128 * i + 256],
                             start=False, stop=False, skip_group_check=True)

        def emit_b(i):
            # odd i: B scores for tile i: keys 128i+64..128i+128, queries
            # j>=64.  For i<NT-1 share the full 128-col stationary with the
            # following emit_a(i+1); extra rows 64:128 garbage, masked.
            Sa = sa_pairs[i // 2]
            base = 192 * (i % 2)
            if i < NT - 1:
                nc.tensor.matmul(Sa[:, base + 128:base + 192],
                                 cur["kT"][:, 128 * i + 64:128 * i + 192],
                                 cur["qT"][:, 128 * i + 64:128 * i + 128],
                                 start=False, stop=(i % 2 == 1),
                                 skip_group_check=True)
            else:
                nc.tensor.matmul(Sa[0:64, base + 128:base + 192],
                                 cur["kT"][:, 128 * i + 64:128 * i + 128],
                                 cur["qT"][:, 128 * i + 64:128 * i + 128],
                                 start=False, stop=True, skip_group_check=True)

        def finish_pair(t):
            # one exp + one mask multiply for tiles (2t, 2t+1)
            Sa = sa_pairs.pop(t)
            p_ab = wpool.tile([128, 384], dt.bfloat16, tag="p_ab")
            nc.scalar.activation(p_ab[:, :], Sa[:, :],
                                 mybir.ActivationFunctionType.Exp, scale=float(SCALE))
            p_abm = wpool.tile([128, 384], dt.bfloat16, tag="p_abm")
            nc.gpsimd.tensor_tensor(p_abm[:, :], p_ab[:, :],
                                    (mABp0 if t == 0 else mABp)[:, :],
                                    mybir.AluOpType.mult)
            for i in (2 * t, 2 * t + 1):
                state[i] = (p_abm, 192 * (i % 2), cur["vsh"], cur["vsr"],
                            cur["vn0"], cur["out_acc"], gstate[i // GT])

        def pv(i):
            p_abm, base, vsh, vsr, vn0, out_acc, pcm = state.pop(i)
            t = i % GT
            O = pout.tile([128, DV], dt.float32, tag="outp")
            if i == 0:
                nc.tensor.matmul(O[:], p_abm[:, 0:128], vn0[:],
                                 start=True, stop=False, skip_group_check=True)
                nc.tensor.matmul(O[:], pcm[:, 0:128], vsr[:],
                                 start=False, stop=True, skip_group_check=True)
            else:
                nc.tensor.matmul(O[:], p_abm[:, base:base + 128],
                                 vsh[:, DV * i:DV * (i + 1)],
                                 start=True, stop=False, skip_group_check=True)
                nc.tensor.matmul(O[:], pcm[:, 128 * t:128 * (t + 1)], vsr[:],
                                 start=False, stop=False, skip_group_check=True)
                # B-part PV widened to a full-array 128-col stationary: p_abm
                # cols base+64:base+128 are entirely masked to 0 for key rows
                # 0:64 (the band j+1<=c<=j+64 is empty there), so output rows
                # 0:64 get +=0; rows 64:128 receive the real B contribution.
                nc.tensor.matmul(O[:], p_abm[0:64, base + 64:base + 192],
                                 vsh[0:64, DV * (i + 1):DV * (i + 2)],
                                 start=False, stop=True, skip_group_check=True)
            rsum = wpool.tile([128, 1], dt.float32, tag="rsum")
            nc.vector.reciprocal(rsum[:], O[:, 128:129])
            nc.vector.tensor_scalar_mul(out_acc[:, 128 * i:128 * (i + 1)],
                                        O[:, 0:128], rsum[:])

        CH = 1024            # qT/kT DMA chunk (columns)
        NCH = S // CH
        VCH = 9 * DV         # vsh DMA chunk (about a quarter)
        OCH = 8 * D          # out DMA chunk (8 tiles)
        out_accs = {}

        def prologue(s):
            # latency-critical tensors on the Scalar DGE queue (short, and
            # nothing else sits ahead of the first activations there); bulk
            # streams on the Sync queue
            kTsr = spool.tile([128, 128], dt.bfloat16, tag="kTsr")
            nc.scalar.dma_start(kTsr[:], kTsr_d[s])
            qT = spool.tile([128, S], dt.bfloat16, tag="qT")
            kT = spool.tile([128, S], dt.bfloat16, tag="kT")
            vsh = spool.tile([128, NVT * DV], dt.bfloat16, tag="vsh")
            nc.sync.dma_start(kT[:, 0:CH], kT_d[s, :, 0:CH])
            nc.sync.dma_start(qT[:, 0:CH], qT_d[s, :, 0:CH])
            if s == 0:
                # group-0 C mask + AB masks must beat the first exp/mask ops
                nc.scalar.dma_start(mCg[:, 0:512], mCg_d[:, 0:512])
                nc.scalar.dma_start(mABp0[:], mABp0_d[:, :])
                nc.scalar.dma_start(mABp[:], mABp_d[:, :])
            nc.sync.dma_start(vsh[:, 0:VCH], vsh_d[s, :, 0:VCH])
            vnr = spool.tile([128, 2 * DV], dt.bfloat16, tag="vnr")
            nc.sync.dma_start(vnr[:], vnr_d[s])
            nc.sync.dma_start(kT[:, CH:S], kT_d[s, :, CH:S])
            nc.sync.dma_start(qT[:, CH:S], qT_d[s, :, CH:S])
            nc.sync.dma_start(vsh[:, VCH:NVT * DV], vsh_d[s, :, VCH:NVT * DV])
            vn0 = vnr[:, 0:DV]
            vsr = vnr[:, DV:2 * DV]
            out_acc = opool.tile([128, NT * D], dt.bfloat16, tag="out_acc")
            out_accs[s] = out_acc
            cur.update(qT=qT, kT=kT, vsh=vsh, kTsr=kTsr, vsr=vsr, vn0=vn0,
                       out_acc=out_acc)

        def out_chunk(s, c):
            nc.sync.dma_start(out_d[s, :, OCH * c:OCH * (c + 1)],
                              out_accs[s][:, OCH * c:OCH * (c + 1)])

        TOT = SLICES * NT
        for tau in range(TOT + LAG):
            if tau < TOT:
                s, i = divmod(tau, NT)
                if i == 0:
                    prologue(s)
                    cgroup(0)
                    emit_a(0)
                    if USE_MERGE:
                        emit_ab(0)
                    else:
                        emit_a(1)
                elif i % 2 == 0:
                    if USE_MERGE:
                        emit_ab(i)
                    else:
                        emit_b(i)
                        emit_a(i + 1)
                else:
                    if i % GT == GT - 1 and i + 1 < NT:
                        cgroup((i + 1) // GT)
                    emit_b(i)
                    if i + 1 < NT:
                        emit_a(i + 1)
                    finish_pair(i // 2)
            if tau == 2:
                # bulk C-mask DMAs queue behind the first activations on the
                # Scalar queue so they can't delay the startup-critical chain
                nc.scalar.dma_start(mCg[:, 512:2048], mCg_d[:, 512:2048])
                nc.scalar.dma_start(mCg[:, 2048:NG * 512], mCg_d[:, 2048:NG * 512])
            if tau >= LAG:
                ps, pi = divmod(tau - LAG, NT)
                pv(pi)
                if pi % 8 == 7:
                    out_chunk(ps, pi // 8)

    nc.finalize()
    _prog_cache["nc"] = nc
    return nc


def _prep_core_inputs(q, k, v, rk, rv, consts):
    """q,k,v: [SLICES, S, D] fp32 for one core; rk, rv: [SLICES, R, D]."""
    mABp, mABp0, mCg = consts
    qb = q.astype(BF16)
    kb = k.astype(BF16)
    vb = v.astype(BF16)
    qT = np.ascontiguousarray(qb.transpose(0, 2, 1))          # [SL, 128, S]
    kT = np.ascontiguousarray(kb.transpose(0, 2, 1))
    # 64-shifted padded v tiles augmented with a ones column, stored
    # per-partition-contiguous: [SL, 128, NVT*DV]; tile j = v rows 128j-64..128j+64
    vpad = np.concatenate([np.zeros((SLICES, 64, D), BF16), vb,
                           np.zeros((SLICES, 64, D), BF16)], axis=1)  # [SL, 4224, D]
    vpad = np.concatenate([vpad, np.ones((SLICES, NVT * 128, 1), BF16)], axis=2)
    vsh = np.ascontiguousarray(
        vpad.reshape(SLICES, NVT, 128, DV).transpose(0, 2, 1, 3).reshape(SLICES, 128, NVT * DV))
    # interleaved strided/relay keys, d-major: col 2s = k[64s], col 2s+1 = rk[s]
    ksr_int = np.empty((SLICES, 128, D), BF16)
    ksr_int[:, 0::2] = kb[:, ::W, :]
    ksr_int[:, 1::2] = rk.astype(BF16)
    kTsr = np.ascontiguousarray(ksr_int.transpose(0, 2, 1))           # [SL, 128, 128]
    # interleaved [str0, rel0, str1, rel1, ...] + ones column
    vsr_pairs = np.empty((SLICES, 128, D), BF16)
    vsr_pairs[:, 0::2] = vb[:, ::W, :]
    vsr_pairs[:, 1::2] = rv.astype(BF16)
    vsr = np.ascontiguousarray(
        np.concatenate([vsr_pairs, np.ones((SLICES, 128, 1), BF16)], axis=2))
    vn0 = np.ascontiguousarray(
        np.concatenate([vb[:, 0:128, :], np.ones((SLICES, 128, 1), BF16)], axis=2))
    vnr = np.ascontiguousarray(np.concatenate([vn0, vsr], axis=2))
    return {
        "qT": qT, "kT": kT, "vsh": vsh, "kTsr": kTsr, "vnr": vnr,
        "mABp": mABp, "mABp0": mABp0, "mCg": mCg,
    }


def make_in_maps(q, k, v, rk, rv):
    consts = _build_consts()
    qf = q.reshape(B * H, S, D)
    kf = k.reshape(B * H, S, D)
    vf = v.reshape(B * H, S, D)
    rkf = rk.reshape(B * H, R, D)
    rvf = rv.reshape(B * H, R, D)
    in_maps = []
    for c in range(NCORES):
        sl = slice(SLICES * c, SLICES * (c + 1))
        in_maps.append(_prep_core_inputs(qf[sl], kf[sl], vf[sl], rkf[sl], rvf[sl],
                                         consts))
    return in_maps


def kernel(q, k, v, rk, rv, _run_kwargs=None):
    q = np.asarray(q, dtype=np.float32)
    k = np.asarray(k, dtype=np.float32)
    v = np.asarray(v, dtype=np.float32)
    rk = np.asarray(rk, dtype=np.float32)
    rv = np.asarray(rv, dtype=np.float32)
    nc = build_program()
    in_maps = make_in_maps(q, k, v, rk, rv)
    res = run_bass_kernel_spmd(nc, in_maps, list(range(NCORES)), **(_run_kwargs or {}))
    out = np.stack([np.asarray(res.results[c]["out"]) for c in range(NCORES)])
    if _run_kwargs:
        kernel.last_results = res
    # out: [NCORES, SLICES, 128, NT*D] -> [B,H,S,D]
    out = out.reshape(B * H, 128, NT, D).transpose(0, 2, 1, 3)
    return out.reshape(B, H, S, D).astype(np.float32)


# revision 34
# speedup vs baseline: 1.1111x; 1.1111x over previous
"""Trainium2 Bass kernel for CronRootAttention (sparse attention).

Shapes (hardcoded): B=2 H=16 S=4096 D=128, W=64, NB=R=64.
Sharding: fused B*H=32 axis split across 8 cores (4 slices/core).

Design (transposed scores + multiplicative masks + paired tiles), per
(b,h) slice, per 128-query tile i (group g = i//4 covers 512 queries):

  scores computed TRANSPOSED: S[key, query] via key-stationary QK
  matmuls so exp(S) is directly the stationary operand for PV (no PE
  transposes anywhere).

  Local scores for a PAIR of tiles (2t, 2t+1) share ONE PSUM bank
  [128, 384] (2x192 fp32 cols = 1536B <= 2KB bank):
    cols   0:128  A(2t):   keys 256t-64 .. 256t+64
    cols 128:192  B(2t):   keys 256t+64 .. 256t+128  (rows 64:128 junk)
    cols 192:320  A(2t+1), cols 320:384  B(2t+1)
  B(i) reuses the identical 128-col stationary of A(i+1) (adjacent in
  the PE queue) so its weight load hides.  Strided/relay scores per
  GROUP in PSUM Sc [128, 512]: one matmul, stationary kTsr (128
  interleaved strided/relay keys), moving qT[:, 512g:512g+512].

  NO additive -1e30 mask matmuls on the PE.  exp() runs on raw scores
  (stale PSUM is bounded: banks are memset once, then only ever hold
  old scores, so exp stays finite) and the band/causal masks are
  applied POST-exp as multiplicative 0/1 bf16 masks: ONE GpSimd
  tensor_tensor per pair ([128,384]), ONE Vector tensor_tensor per
  group ([128,512]).  Zeroed p rows contribute nothing to PV numerator
  or the ones-column denominator.

  PV per tile (3 matmuls into O [128, 129]): p regions stationary,
  moving v tiles carry a ones-column so O[:,128] = softmax denominator.
  The B-part PV uses the widened stationary p_abm[0:64, base+64:
  base+192] (full-array 128-col load; the extra columns are exactly the
  always-masked part of the A band so output rows 0:64 get +=0).
  DVE reciprocal + per-partition scale -> bf16 out column block; output
  DMA per 8 tiles from a contiguous [128, NT*128] SBUF accumulator
  (dram layout [slice, partition, tile*128+d]; host transposes).

  PSUM banks: Sa-pairs x3 + Sc x2 + O x3 = 8.

  Schedule: ONE flat software pipeline across all 4 slices (no
  inter-slice seams, keeps the PE p-state ramped): per step emit
  B(i)+A(i+1) (+ Cq at group boundaries), finish_pair on odd steps,
  PV lagging LAG=4 steps.  DMA: bulk streams chunked on the Sync DGE
  queue; latency-critical kTsr/masks on the Scalar DGE queue; the big
  C-mask tail is emitted late so it queues behind the first
  activations; input chunk 0 sized so compute starts ~2 DMAs in.
"""

import numpy as np
import ml_dtypes

import concourse.bass as bass
import concourse.bacc as bacc
import concourse.tile as tile
from concourse import mybir
from concourse.bass_utils import run_bass_kernel_spmd

BF16 = ml_dtypes.bfloat16
B, H, S, D = 2, 16, 4096, 128
W = 64
NB = S // W          # 64
R = NB               # 64
NCORES = 8
SLICES = B * H // NCORES   # 4
NT = S // 128        # 32 query tiles per slice
GT = 4               # tiles per strided-score group
NG = NT // GT        # 8 groups per slice
SCALE = 1.0 / np.sqrt(np.float32(D))
DV = D + 1           # v columns + ones column
NVT = S // 128 + 1   # 33 shifted v tiles

_prog_cache = {}
USE_MERGE = False
LAG = 4
WBUFS = 5
GBUFS = 3


def _build_consts():
    c = np.arange(128)[:, None]   # partition = key index within region
    j = np.arange(128)[None, :]   # col = query index within tile
    # AB mask (i>=1), multiplicative:
    #  A cols 0:128: key = 128i-64+c, query m = 128i+j: valid j+1<=c<=j+64
    #  B cols 128:192: key = 128i+64+c (c<64), query j'=j-64: valid c<=j'
    mA = ((c >= j + 1) & (c <= j + 64)).astype(np.float32)
    j2 = np.arange(64)[None, :]
    mB = ((c < 64) & (c <= j2)).astype(np.float32)
    mAB = np.concatenate([mA, mB], axis=1)             # [128, 192]
    # i=0 variant: key = c, query m = j: valid j-63<=c<=j; B region zero
    mAB0 = np.concatenate(
        [((c <= j) & (c >= j - 63)).astype(np.float32),
         np.zeros((128, 64), np.float32)], axis=1)
    # paired masks for the 2-tile Sa banks
    mABp = np.concatenate([mAB, mAB], axis=1)          # [128, 384]
    mABp0 = np.concatenate([mAB0, mAB], axis=1)
    # C masks per group g: [128, 512]; row 2s = strided key s (pos 64s),
    # row 2s+1 = relay s (block end 64s+63); query m = 512g + q.
    # valid strided: 64s < max(m-63,0); valid relay: 64s+63 < max(m-63,0)
    mC = np.zeros((NG, 128, 512), np.float32)
    s_ = np.arange(64)[:, None]
    for g in range(NG):
        m = (512 * g + np.arange(512))[None, :]
        ls = np.maximum(m - 63, 0)
        mC[g, 0::2, :] = (64 * s_ < ls).astype(np.float32)
        mC[g, 1::2, :] = (64 * s_ + 63 < ls).astype(np.float32)
    mCg = mC.transpose(1, 0, 2).reshape(128, NG * 512)  # [128, 8*512]
    return (mABp.astype(BF16), mABp0.astype(BF16),
            np.ascontiguousarray(mCg).astype(BF16))


def build_program():
    if "nc" in _prog_cache:
        return _prog_cache["nc"]
    dt = mybir.dt
    nc = bacc.Bacc("TRN2", target_bir_lowering=False, debug=False)

    qT_d = nc.declare_dram_parameter("qT", [SLICES, 128, S], dt.bfloat16, isOutput=False)
    kT_d = nc.declare_dram_parameter("kT", [SLICES, 128, S], dt.bfloat16, isOutput=False)
    vsh_d = nc.declare_dram_parameter("vsh", [SLICES, 128, NVT * DV], dt.bfloat16, isOutput=False)
    kTsr_d = nc.declare_dram_parameter("kTsr", [SLICES, 128, 128], dt.bfloat16, isOutput=False)
    vnr_d = nc.declare_dram_parameter("vnr", [SLICES, 128, 2 * DV], dt.bfloat16, isOutput=False)
    mABp_d = nc.declare_dram_parameter("mABp", [128, 384], dt.bfloat16, isOutput=False)
    mABp0_d = nc.declare_dram_parameter("mABp0", [128, 384], dt.bfloat16, isOutput=False)
    mCg_d = nc.declare_dram_parameter("mCg", [128, NG * 512], dt.bfloat16, isOutput=False)
    # out stored [slice, partition(=query%128), tile*128+d]; host transposes
    out_d = nc.declare_dram_parameter("out", [SLICES, 128, NT * D], dt.bfloat16, isOutput=True)

    from contextlib import ExitStack
    with tile.TileContext(nc) as tc, ExitStack() as ctx:
        cpool = ctx.enter_context(tc.tile_pool(name="consts", bufs=1))
        # paired AB masks: [tile 2t | tile 2t+1], 384 cols each
        mABp = cpool.tile([128, 384], dt.bfloat16, tag="mABp")
        mABp0 = cpool.tile([128, 384], dt.bfloat16, tag="mABp0")
        mCg = cpool.tile([128, NG * 512], dt.bfloat16, tag="mCg")
        # const DMAs are issued inside slice 0's prologue (critical first)

        spool = ctx.enter_context(tc.tile_pool(name="slice_in", bufs=2))
        pscores = ctx.enter_context(tc.tile_pool(name="pscores", bufs=3, space="PSUM"))
        pcpool = ctx.enter_context(tc.tile_pool(name="pcscores", bufs=2, space="PSUM"))
        pout = ctx.enter_context(tc.tile_pool(name="pout", bufs=3, space="PSUM"))
        wpool = ctx.enter_context(tc.tile_pool(name="work", bufs=WBUFS))
        gpool = ctx.enter_context(tc.tile_pool(name="gwork", bufs=GBUFS))
        opool = ctx.enter_context(tc.tile_pool(name="outacc", bufs=2))

        # one-time: clear the Sa banks so first-use stale PSUM can't be huge
        for z in range(3):
            zt = pscores.tile([128, 384], dt.float32, tag="scores")
            nc.vector.memset(zt[:], 0.0)

        state = {}
        gstate = {}
        cur = {}

        def cgroup(g):
            Sc = pcpool.tile([128, 512], dt.float32, tag="cscores")
            nc.tensor.matmul(Sc[:, :], cur["kTsr"][:, 0:128],
                             cur["qT"][:, 512 * g:512 * (g + 1)],
                             start=True, stop=True, skip_group_check=True)
            pc = gpool.tile([128, 512], dt.bfloat16, tag="pc")
            nc.scalar.activation(pc[:, :], Sc[:, :],
                                 mybir.ActivationFunctionType.Exp, scale=float(SCALE))
            pcm = gpool.tile([128, 512], dt.bfloat16, tag="pcm")
            nc.vector.tensor_tensor(pcm[:, :], pc[:, :],
                                    mCg[:, 512 * g:512 * (g + 1)],
                                    mybir.AluOpType.mult)
            gstate[g] = pcm

        sa_pairs = {}

        def emit_a(i):
            # A scores for tile i into the pair bank (pair = tiles 2t,2t+1
            # sharing one PSUM bank: tile cols at 192*(i&1)).  i>=1 shares
            # its stationary kT[:, 128i-64:128i+64] with the preceding
            # emit_b(i-1) (identical weights AP back-to-back).
            if i % 2 == 0:
                Sa = pscores.tile([128, 384], dt.float32, tag="scores")
                sa_pairs[i // 2] = Sa
            Sa = sa_pairs[i // 2]
            base = 192 * (i % 2)
            qTi = cur["qT"][:, 128 * i:128 * (i + 1)]
            kTa = cur["kT"][:, 0:128] if i == 0 else \
                cur["kT"][:, 128 * i - 64:128 * i + 64]
            # start=True only on the pair's first write (clears bank)
            nc.tensor.matmul(Sa[:, base:base + 128], kTa, qTi,
                             start=(i % 2 == 0), stop=False,
                             skip_group_check=True)

        def emit_ab(i):
            # even i: ONE matmul for B_i + A_{i+1} (one weight load).  Their
            # movings are adjacent qT cols 128i+64:128i+256 and the outputs
            # are adjacent PSUM cols 128:320 of the shared pair bank.  Output
            # cols 128:192 = B_i (rows 64:128 garbage, masked post-exp); for
            # i=0 the whole B region is garbage (masked by mABp0).
            Sa = sa_pairs[i // 2]
            nc.tensor.matmul(Sa[:, 128:320],
                             cur["kT"][:, 128 * i + 64:128 * i + 192],
                             cur["qT"][:, 128 * i + 64:128 * i + 256],
                             start=False, stop=False, skip_group_check=True)

        def emit_b(i):
            # odd i: B scores for tile i: keys 128i+64..128i+128, queries
            # j>=64.  For i<NT-1 share the full 128-col stationary with the
            # following emit_a(i+1); extra rows 64:128 garbage, masked.
            Sa = sa_pairs[i // 2]
            base = 192 * (i % 2)
            if i < NT - 1:
                nc.tensor.matmul(Sa[:, base + 128:base + 192],
                                 cur["kT"][:, 128 * i + 64:128 * i + 192],
                                 cur["qT"][:, 128 * i + 64:128 * i + 128],
                                 start=False, stop=(i % 2 == 1),
                                 skip_group_check=True)
            else:
                nc.tensor.matmul(Sa[0:64, base + 128:base + 192],
                                 cur["kT"][:, 128 * i + 64:128 * i + 128],
                                 cur["qT"][:, 128 * i + 64:128 * i + 128],
                                 start=False, stop=True, skip_group_check=True)

        def finish_pair(t):
            # one exp + one mask multiply for tiles (2t, 2t+1)
            Sa = sa_pairs.pop(t)
            p_ab = wpool.tile([128, 384], dt.bfloat16, tag="p_ab")
            nc.scalar.activation(p_ab[:, :], Sa[:, :],
                                 mybir.ActivationFunctionType.Exp, scale=float(SCALE))
            p_abm = wpool.tile([128, 384], dt.bfloat16, tag="p_abm")
            nc.gpsimd.tensor_tensor(p_abm[:, :], p_ab[:, :],
                                    (mABp0 if t == 0 else mABp)[:, :],
                                    mybir.AluOpType.mult)
            for i in (2 * t, 2 * t + 1):
                state[i] = (p_abm, 192 * (i % 2), cur["vsh"], cur["vsr"],
                            cur["vn0"], cur["out_acc"], gstate[i // GT])

        def pv(i):
            p_abm, base, vsh, vsr, vn0, out_acc, pcm = state.pop(i)
            t = i % GT
            O = pout.tile([128, DV], dt.float32, tag="outp")
            if i == 0:
                nc.tensor.matmul(O[:], p_abm[:, 0:128], vn0[:],
                                 start=True, stop=False, skip_group_check=True)
                nc.tensor.matmul(O[:], pcm[:, 0:128], vsr[:],
                                 start=False, stop=True, skip_group_check=True)
            else:
                nc.tensor.matmul(O[:], p_abm[:, base:base + 128],
                                 vsh[:, DV * i:DV * (i + 1)],
                                 start=True, stop=False, skip_group_check=True)
                nc.tensor.matmul(O[:], pcm[:, 128 * t:128 * (t + 1)], vsr[:],
                                 start=False, stop=False, skip_group_check=True)
                # B-part PV widened to a full-array 128-col stationary: p_abm
                # cols base+64:base+128 are entirely masked to 0 for key rows
                # 0:64 (the band j+1<=c<=j+64 is empty there), so output rows
                # 0:64 get +=0; rows 64:128 receive the real B contribution.
                nc.tensor.matmul(O[:], p_abm[0:64, base + 64:base + 192],
                                 vsh[0:64, DV * (i + 1):DV * (i + 2)],
                                 start=False, stop=True, skip_group_check=True)
            rsum = wpool.tile([128, 1], dt.float32, tag="rsum")
            nc.vector.reciprocal(rsum[:], O[:, 128:129])
            nc.vector.tensor_scalar_mul(out_acc[:, 128 * i:128 * (i + 1)],
                                        O[:, 0:128], rsum[:])

        CH = 1024            # qT/kT DMA chunk (columns)
        NCH = S // CH
        VCH = 9 * DV         # vsh DMA chunk (about a quarter)
        OCH = 8 * D          # out DMA chunk (8 tiles)
        out_accs = {}

        def prologue(s):
            # latency-critical tensors on the Scalar DGE queue (short, and
            # nothing else sits ahead of the first activations there); bulk
            # streams on the Sync queue
            kTsr = spool.tile([128, 128], dt.bfloat16, tag="kTsr")
            nc.scalar.dma_start(kTsr[:], kTsr_d[s])
            qT = spool.tile([128, S], dt.bfloat16, tag="qT")
            kT = spool.tile([128, S], dt.bfloat16, tag="kT")
            vsh = spool.tile([128, NVT * DV], dt.bfloat16, tag="vsh")
            nc.sync.dma_start(kT[:, 0:CH], kT_d[s, :, 0:CH])
            nc.sync.dma_start(qT[:, 0:CH], qT_d[s, :, 0:CH])
            if s == 0:
                # group-0 C mask + AB masks must beat the first exp/mask ops
                nc.scalar.dma_start(mCg[:, 0:512], mCg_d[:, 0:512])
                nc.scalar.dma_start(mABp0[:], mABp0_d[:, :])
                nc.scalar.dma_start(mABp[:], mABp_d[:, :])
            nc.sync.dma_start(vsh[:, 0:VCH], vsh_d[s, :, 0:VCH])
            vnr = spool.tile([128, 2 * DV], dt.bfloat16, tag="vnr")
            nc.sync.dma_start(vnr[:], vnr_d[s])
            nc.sync.dma_start(kT[:, CH:S], kT_d[s, :, CH:S])
            nc.sync.dma_start(qT[:, CH:S], qT_d[s, :, CH:S])
            nc.sync.dma_start(vsh[:, VCH:NVT * DV], vsh_d[s, :, VCH:NVT * DV])
            vn0 = vnr[:, 0:DV]
            vsr = vnr[:, DV:2 * DV]
            out_acc = opool.tile([128, NT * D], dt.bfloat16, tag="out_acc")
            out_accs[s] = out_acc
            cur.update(qT=qT, kT=kT, vsh=vsh, kTsr=kTsr, vsr=vsr, vn0=vn0,
                       out_acc=out_acc)

        def out_chunk(s, c):
            nc.sync.dma_start(out_d[s, :, OCH * c:OCH * (c + 1)],
                              out_accs[s][:, OCH * c:OCH * (c + 1)])

        TOT = SLICES * NT
        for tau in range(TOT + LAG):
            if tau < TOT:
                s, i = divmod(tau, NT)
                if i == 0:
                    prologue(s)
                    cgroup(0)
                    emit_a(0)
                    if USE_MERGE:
                        emit_ab(0)
                    else:
                        emit_a(1)
                elif i % 2 == 0:
                    if USE_MERGE:
                        emit_ab(i)
                    else:
                        emit_b(i)
                        emit_a(i + 1)
                else:
                    if i % GT == GT - 1 and i + 1 < NT:
                        cgroup((i + 1) // GT)
                    emit_b(i)
                    if i + 1 < NT:
                        emit_a(i + 1)
                    finish_pair(i // 2)
            if tau == 2:
                # bulk C-mask DMAs queue behind the first activations on the
                # Scalar queue so they can't delay the startup-critical chain
                nc.scalar.dma_start(mCg[:, 512:2048], mCg_d[:, 512:2048])
                nc.scalar.dma_start(mCg[:, 2048:NG * 512], mCg_d[:, 2048:NG * 512])
            if tau >= LAG:
                ps, pi = divmod(tau - LAG, NT)
                pv(pi)
                if pi % 8 == 7:
                    out_chunk(ps, pi // 8)

    nc.finalize()
    _prog_cache["nc"] = nc
    return nc


def _prep_core_inputs(q, k, v, rk, rv, consts):
    """q,k,v: [SLICES, S, D] fp32 for one core; rk, rv: [SLICES, R, D]."""
    mABp, mABp0, mCg = consts
    qb = q.astype(BF16)
    kb = k.astype(BF16)
    vb = v.astype(BF16)
    qT = np.ascontiguousarray(qb.transpose(0, 2, 1))          # [SL, 128, S]
    kT = np.ascontiguousarray(kb.transpose(0, 2, 1))
    # 64-shifted padded v tiles augmented with a ones column, stored
    # per-partition-contiguous: [SL, 128, NVT*DV]; tile j = v rows 128j-64..128j+64
    vpad = np.concatenate([np.zeros((SLICES, 64, D), BF16), vb,
                           np.zeros((SLICES, 64, D), BF16)], axis=1)  # [SL, 4224, D]
    vpad = np.concatenate([vpad, np.ones((SLICES, NVT * 128, 1), BF16)], axis=2)
    vsh = np.ascontiguousarray(
        vpad.reshape(SLICES, NVT, 128, DV).transpose(0, 2, 1, 3).reshape(SLICES, 128, NVT * DV))
    # interleaved strided/relay keys, d-major: col 2s = k[64s], col 2s+1 = rk[s]
    ksr_int = np.empty((SLICES, 128, D), BF16)
    ksr_int[:, 0::2] = kb[:, ::W, :]
    ksr_int[:, 1::2] = rk.astype(BF16)
    kTsr = np.ascontiguousarray(ksr_int.transpose(0, 2, 1))           # [SL, 128, 128]
    # interleaved [str0, rel0, str1, rel1, ...] + ones column
    vsr_pairs = np.empty((SLICES, 128, D), BF16)
    vsr_pairs[:, 0::2] = vb[:, ::W, :]
    vsr_pairs[:, 1::2] = rv.astype(BF16)
    vsr = np.ascontiguousarray(
        np.concatenate([vsr_pairs, np.ones((SLICES, 128, 1), BF16)], axis=2))
    vn0 = np.ascontiguousarray(
        np.concatenate([vb[:, 0:128, :], np.ones((SLICES, 128, 1), BF16)], axis=2))
    vnr = np.ascontiguousarray(np.concatenate([vn0, vsr], axis=2))
    return {
        "qT": qT, "kT": kT, "vsh": vsh, "kTsr": kTsr, "vnr": vnr,
        "mABp": mABp, "mABp0": mABp0, "mCg": mCg,
    }


def make_in_maps(q, k, v, rk, rv):
    consts = _build_consts()
    qf = q.reshape(B * H, S, D)
    kf = k.reshape(B * H, S, D)
    vf = v.reshape(B * H, S, D)
    rkf = rk.reshape(B * H, R, D)
    rvf = rv.reshape(B * H, R, D)
    in_maps = []
    for c in range(NCORES):
        sl = slice(SLICES * c, SLICES * (c + 1))
        in_maps.append(_prep_core_inputs(qf[sl], kf[sl], vf[sl], rkf[sl], rvf[sl],
                                         consts))
    return in_maps


def kernel(q, k, v, rk, rv, _run_kwargs=None):
    q = np.asarray(q, dtype=np.float32)
    k = np.asarray(k, dtype=np.float32)
    v = np.asarray(v, dtype=np.float32)
    rk = np.asarray(rk, dtype=np.float32)
    rv = np.asarray(rv, dtype=np.float32)
    nc = build_program()
    in_maps = make_in_maps(q, k, v, rk, rv)
    res = run_bass_kernel_spmd(nc, in_maps, list(range(NCORES)), **(_run_kwargs or {}))
    out = np.stack([np.asarray(res.results[c]["out"]) for c in range(NCORES)])
    if _run_kwargs:
        kernel.last_results = res
    # out: [NCORES, SLICES, 128, NT*D] -> [B,H,S,D]
    out = out.reshape(B * H, 128, NT, D).transpose(0, 2, 1, 3)
    return out.reshape(B, H, S, D).astype(np.float32)
